# revision 1
# baseline (speedup 1.0000x reference)
"""CosineSimCodebook (VQ) Trainium2 kernel, 8 NeuronCores.

Strategy:
  Phase 1 (token-data-parallel): each core takes 4096 tokens, computes
    dist = x @ embed^T via split-bf16 (hi*hi + lo*hi + hi*lo) matmuls on PE
    (near-fp32 accuracy at 3x bf16 cost), argmax via DVE Max8/MaxIndex,
    quantize via indirect-DMA gather of codebook rows.
  Host: sorts tokens by assigned cluster, packs cluster-aligned 128-token
    tiles per 512-cluster shard (pure index routing, no arithmetic).
  Phase 2 (cluster-sharded): each core gathers its shard's token rows,
    merges same-cluster rows with a selection-matrix matmul, scatters rows
    into an embed-sum table, then applies the EMA update + l2norm.
    (cs_smoothed cancels inside l2norm, so laplace smoothing drops out of
    new_embed exactly; new_cluster_size does not depend on it.)
"""

import sys

sys.path.insert(0, "/opt/trn_rl_repo")

import numpy as np
import ml_dtypes

import concourse.bass as bass
import concourse.mybir as mybir
import concourse.tile as tile
from concourse import bacc
from concourse.bass_utils import run_bass_kernel_spmd
from concourse.masks import make_identity

N_CORES = 8
B, N, D = 8, 4096, 512
C = 4096
TOK = B * N                  # 32768 tokens
TPC = TOK // N_CORES         # 4096 tokens per core (phase 1)
VPC = C // N_CORES           # 512 clusters per core (phase 2)
NTILES1 = TPC // 128         # 32 token tiles per core
CCH = C // 512               # 8 psum chunks of 512 clusters
KT = 12                      # 3 split terms x 4 k-tiles of 128
NT2 = 40                     # phase-2 token tiles per core (fixed, padded)
G2 = VPC // 128              # 4 cluster row groups
DECAY = 0.8
P = 128
F32 = mybir.dt.float32
BF16 = mybir.dt.bfloat16
I32 = mybir.dt.int32
U32 = mybir.dt.uint32

_cache = {}
LAST_RESULTS = {}


def _build_phase1():
    nc = bacc.Bacc("TRN2", target_bir_lowering=False, debug=False,
                   num_devices=N_CORES)
    xt_d = nc.dram_tensor("xt", [KT * P, TPC], BF16, kind="ExternalInput")
    et_d = nc.dram_tensor("et", [KT * P, C], BF16, kind="ExternalInput")
    emb_d = nc.dram_tensor("emb", [C, D], F32, kind="ExternalInput")
    dist_d = nc.dram_tensor("dist", [TPC, C], F32, kind="ExternalOutput")
    ind_d = nc.dram_tensor("ind", [TPC, 1], I32, kind="ExternalOutput")
    quant_d = nc.dram_tensor("quant", [TPC, D], F32, kind="ExternalOutput")

    xt_r = xt_d.ap().rearrange("(k p) t -> p k t", p=P)
    et_r = et_d.ap().rearrange("(k p) c -> p k c", p=P)

    with tile.TileContext(nc) as tc:
        with (
            tc.tile_pool(name="resident", bufs=1) as res_pool,
            tc.tile_pool(name="xt", bufs=3) as xt_pool,
            tc.tile_pool(name="dist", bufs=2) as dist_pool,
            tc.tile_pool(name="small", bufs=3) as small_pool,
            tc.tile_pool(name="psum", bufs=8, space="PSUM") as psum_pool,
        ):
            et_s = res_pool.tile([P, KT, C], BF16)
            nc.sync.dma_start(et_s[:], et_r)
            for t in range(NTILES1):
                xt_s = xt_pool.tile([P, KT, P], BF16, tag="xt")
                nc.sync.dma_start(xt_s[:], xt_r[:, :, t * P:(t + 1) * P])
                dist_s = dist_pool.tile([P, C], F32, tag="dist")
                for cc in range(CCH):
                    ps = psum_pool.tile([P, 512], F32, tag="ps")
                    for k in range(KT):
                        nc.tensor.matmul(
                            ps[:], xt_s[:, k],
                            et_s[:, k, cc * 512:(cc + 1) * 512],
                            start=(k == 0), stop=(k == KT - 1))
                    nc.scalar.copy(dist_s[:, cc * 512:(cc + 1) * 512], ps[:])
                nc.sync.dma_start(dist_d.ap()[t * P:(t + 1) * P], dist_s[:])
                mx = small_pool.tile([P, 8], F32, tag="mx")
                idx = small_pool.tile([P, 8], U32, tag="idx")
                nc.vector.max(out=mx[:], in_=dist_s[:])
                nc.vector.max_index(out=idx[:], in_max=mx[:], in_values=dist_s[:])
                idx32 = small_pool.tile([P, 1], I32, tag="idx32")
                nc.vector.tensor_copy(idx32[:], idx[:, 0:1])
                nc.sync.dma_start(ind_d.ap()[t * P:(t + 1) * P], idx32[:])
                q = small_pool.tile([P, D], F32, tag="q")
                nc.gpsimd.indirect_dma_start(
                    out=q[:], out_offset=None, in_=emb_d.ap(),
                    in_offset=bass.IndirectOffsetOnAxis(ap=idx32[:, :1], axis=0))
                nc.sync.dma_start(quant_d.ap()[t * P:(t + 1) * P], q[:])
    nc.compile()
    return nc


def _build_phase2():
    nc = bacc.Bacc("TRN2", target_bir_lowering=False, debug=False,
                   num_devices=N_CORES)
    x_d = nc.dram_tensor("x", [TOK, D], F32, kind="ExternalInput")
    gidx_d = nc.dram_tensor("gidx", [NT2, P, 1], I32, kind="ExternalInput")
    grid_d = nc.dram_tensor("grid", [NT2, P, 1], I32, kind="ExternalInput")
    cs_d = nc.dram_tensor("cs", [VPC], F32, kind="ExternalInput")
    ea_d = nc.dram_tensor("ea", [VPC, D], F32, kind="ExternalInput")
    table_d = nc.dram_tensor("table", [VPC + P, D + 1], F32)
    ne_d = nc.dram_tensor("ne", [VPC, D], F32, kind="ExternalOutput")
    ncs_d = nc.dram_tensor("ncs", [VPC], F32, kind="ExternalOutput")

    with tile.TileContext(nc) as tc:
        with (
            tc.tile_pool(name="sbuf", bufs=3) as pool,
            tc.tile_pool(name="big", bufs=1) as big,
            tc.tile_pool(name="once", bufs=1) as once,
            tc.tile_pool(name="psum", bufs=4, space="PSUM") as psum,
        ):
            ident = once.tile([P, P], F32)
            make_identity(nc, ident[:])
            zero_t = once.tile([P, D + 1], F32)
            nc.vector.memset(zero_t[:], 0.0)
            for v0 in range(0, VPC + P, P):
                nc.sync.dma_start(table_d.ap()[v0:v0 + P], zero_t[:])
            for t in range(NT2):
                tok = pool.tile([P, 1], I32, tag="tok")
                nc.sync.dma_start(tok[:], gidx_d.ap()[t])
                xg = pool.tile([P, D], F32, tag="xg")
                nc.gpsimd.indirect_dma_start(
                    out=xg[:], out_offset=None, in_=x_d.ap(),
                    in_offset=bass.IndirectOffsetOnAxis(ap=tok[:, :1], axis=0))
                cid = pool.tile([P, 1], I32, tag="cid")
                nc.sync.dma_start(cid[:], grid_d.ap()[t])
                cid_f = pool.tile([P, 1], F32, tag="cidf")
                nc.vector.tensor_copy(cid_f[:], cid[:])
                cidT_ps = psum.tile([P, P], F32, tag="cidT")
                nc.tensor.transpose(out=cidT_ps[:],
                                    in_=cid_f[:].to_broadcast([P, P]),
                                    identity=ident[:])
                cidT = pool.tile([P, P], F32, tag="cidT_s")
                nc.vector.tensor_copy(cidT[:], cidT_ps[:])
                sel = pool.tile([P, P], F32, tag="sel")
                nc.vector.tensor_tensor(out=sel[:],
                                        in0=cid_f[:].to_broadcast([P, P]),
                                        in1=cidT[:], op=mybir.AluOpType.is_equal)
                cnt = pool.tile([P, 1], F32, tag="cnt")
                nc.vector.reduce_sum(cnt[:], sel[:], axis=mybir.AxisListType.X)
                comb_ps = psum.tile([P, D], F32, tag="comps")
                nc.tensor.matmul(comb_ps[:], sel[:], xg[:], start=True, stop=True)
                comb = pool.tile([P, D + 1], F32, tag="comb")
                nc.scalar.copy(comb[:, :D], comb_ps[:])
                nc.vector.tensor_copy(comb[:, D:D + 1], cnt[:])
                nc.gpsimd.indirect_dma_start(
                    out=table_d.ap(),
                    out_offset=bass.IndirectOffsetOnAxis(ap=cid[:, :1], axis=0),
                    in_=comb[:], in_offset=None)
            # EMA + l2norm over the 512-cluster shard
            tbl = big.tile([P, G2, D + 1], F32)
            nc.sync.dma_start(tbl[:],
                              table_d.ap()[:VPC].rearrange("(g p) c -> p g c", p=P))
            cs_s = pool.tile([P, G2], F32, tag="cs")
            nc.sync.dma_start(cs_s[:], cs_d.ap().rearrange("(g p) -> p g", p=P))
            ea_s = big.tile([P, G2, D], F32)
            nc.sync.dma_start(ea_s[:], ea_d.ap().rearrange("(g p) d -> p g d", p=P))
            ncs_s = pool.tile([P, G2], F32, tag="ncs")
            nc.vector.tensor_scalar_mul(ncs_s[:], cs_s[:], DECAY)
            cnt_sc = pool.tile([P, G2], F32, tag="cnts")
            nc.vector.tensor_scalar_mul(cnt_sc[:], tbl[:, :, D], 1.0 - DECAY)
            nc.vector.tensor_add(ncs_s[:], ncs_s[:], cnt_sc[:])
            nc.sync.dma_start(ncs_d.ap().rearrange("(g p) -> p g", p=P), ncs_s[:])
            nea = big.tile([P, G2, D], F32)
            nc.vector.tensor_scalar_mul(nea[:], ea_s[:], DECAY)
            tbl_sc = big.tile([P, G2, D], F32)
            nc.vector.tensor_scalar_mul(tbl_sc[:], tbl[:, :, :D], 1.0 - DECAY)
            nc.vector.tensor_add(nea[:], nea[:], tbl_sc[:])
            sq = big.tile([P, G2, D], F32)
            nc.vector.tensor_mul(sq[:], nea[:], nea[:])
            ss = pool.tile([P, G2], F32, tag="ss")
            nc.vector.reduce_sum(ss[:], sq[:], axis=mybir.AxisListType.X)
            nrm = pool.tile([P, G2], F32, tag="nrm")
            nc.scalar.sqrt(nrm[:], ss[:])
            nc.vector.tensor_scalar_max(nrm[:], nrm[:], 1e-12)
            inv = pool.tile([P, G2], F32, tag="inv")
            nc.vector.reciprocal(inv[:], nrm[:])
            ne_s = big.tile([P, G2, D], F32)
            nc.vector.tensor_tensor(out=ne_s[:], in0=nea[:],
                                    in1=inv[:, :, None].to_broadcast([P, G2, D]),
                                    op=mybir.AluOpType.mult)
            nc.sync.dma_start(ne_d.ap().rearrange("(g p) d -> p g d", p=P), ne_s[:])
    nc.compile()
    return nc


def _split3(a):
    """fp32 [K, M] -> bf16 [3K, M] rows: (hi, lo, hi)."""
    hi = a.astype(ml_dtypes.bfloat16)
    lo = (a - hi.astype(np.float32)).astype(ml_dtypes.bfloat16)
    return hi, lo


def _pack_shard(tok_sorted, clus_sorted, vlo):
    """Pack shard tokens (sorted by cluster) into NT2 cluster-aligned tiles."""
    gidx = np.zeros((NT2, P), np.int32)
    grid = np.full((NT2, P), VPC, np.int32)  # dummy row
    ti, pos, i, n = 0, 0, 0, len(tok_sorted)
    while i < n:
        j = i
        while j < n and clus_sorted[j] == clus_sorted[i]:
            j += 1
        cnt = j - i
        assert cnt <= P, f"cluster with {cnt} tokens in one shard"
        if pos + cnt > P:
            ti += 1
            pos = 0
        assert ti < NT2, "NT2 too small"
        gidx[ti, pos:pos + cnt] = tok_sorted[i:j]
        grid[ti, pos:pos + cnt] = clus_sorted[i] - vlo
        pos += cnt
        if pos == P:
            ti += 1
            pos = 0
        i = j
    return gidx.reshape(NT2, P, 1), grid.reshape(NT2, P, 1)


def kernel(x, embed, cluster_size, embed_avg):
    x = np.asarray(x, np.float32)
    embed = np.asarray(embed, np.float32)
    cluster_size = np.asarray(cluster_size, np.float32)
    embed_avg = np.asarray(embed_avg, np.float32)

    trace = bool(LAST_RESULTS.get("trace"))
    if "p1" not in _cache:
        _cache["p1"] = _build_phase1()
    if "p2" not in _cache:
        _cache["p2"] = _build_phase2()

    X = np.ascontiguousarray(x.reshape(TOK, D))
    E = np.ascontiguousarray(embed[0])

    xh, xl = _split3(np.ascontiguousarray(X.T))
    eh, el = _split3(np.ascontiguousarray(E.T))
    xt_full = np.concatenate([xh, xl, xh], 0)       # [1536, TOK]
    et_np = np.ascontiguousarray(np.concatenate([eh, eh, el], 0))  # [1536, C]

    in_maps1 = []
    for s in range(N_CORES):
        sl = slice(s * TPC, (s + 1) * TPC)
        in_maps1.append({
            "xt": np.ascontiguousarray(xt_full[:, sl]),
            "et": et_np,
            "emb": E,
        })
    r1 = run_bass_kernel_spmd(_cache["p1"], in_maps1, list(range(N_CORES)),
                              trace=trace)
    LAST_RESULTS["p1"] = r1

    ind_full = np.concatenate(
        [r1.results[s]["ind"][:, 0] for s in range(N_CORES)])  # [TOK] int32

    order = np.argsort(ind_full, kind="stable").astype(np.int32)
    sorted_clus = ind_full[order]
    bounds = np.searchsorted(sorted_clus, np.arange(0, C + 1, VPC))
    in_maps2 = []
    for s in range(N_CORES):
        lo, hi = bounds[s], bounds[s + 1]
        gidx, grid = _pack_shard(order[lo:hi], sorted_clus[lo:hi], s * VPC)
        in_maps2.append({
            "x": X,
            "gidx": gidx,
            "grid": grid,
            "cs": np.ascontiguousarray(cluster_size[0, s * VPC:(s + 1) * VPC]),
            "ea": np.ascontiguousarray(embed_avg[0, s * VPC:(s + 1) * VPC]),
        })
    r2 = run_bass_kernel_spmd(_cache["p2"], in_maps2, list(range(N_CORES)),
                              trace=trace)
    LAST_RESULTS["p2"] = r2

    quantize = np.concatenate(
        [r1.results[s]["quant"] for s in range(N_CORES)]).reshape(B, N, D)
    embed_ind = ind_full.reshape(B, N).astype(np.int32)
    dist = np.concatenate(
        [r1.results[s]["dist"] for s in range(N_CORES)]).reshape(1, B, N, C)
    new_embed = np.concatenate(
        [r2.results[s]["ne"] for s in range(N_CORES)])[None]  # [1, C, D]
    new_cluster_size = np.concatenate(
        [r2.results[s]["ncs"] for s in range(N_CORES)])[None]  # [1, C]
    return quantize, embed_ind, dist, new_embed, new_cluster_size


# revision 3
# speedup vs baseline: 1.3712x; 1.3712x over previous
"""CosineSimCodebook (VQ) Trainium2 kernel, 8 NeuronCores.

Strategy:
  Phase 1 (token-data-parallel): each core takes 4096 tokens, computes
    dist = x @ embed^T via split-bf16 (hi*hi + lo*hi + hi*lo) matmuls on PE
    (near-fp32 accuracy at 3x bf16 cost), argmax via DVE Max8/MaxIndex,
    quantize via indirect-DMA gather of codebook rows.
  Host: sorts tokens by assigned cluster and builds routing metadata
    (gather lists + 0/1 selection matrices) — no arithmetic on host.
  Phase 2 (cluster-sharded): each core gathers its shard's token rows,
    sums same-cluster rows with a selection matmul into a dense per-tile
    window, stages windows to DRAM, gathers them back in cluster order,
    then applies the EMA update + l2norm.
    (cs_smoothed cancels inside l2norm, so laplace smoothing drops out of
    new_embed exactly; new_cluster_size does not depend on it.)
"""

import sys

sys.path.insert(0, "/opt/trn_rl_repo")

import numpy as np
import ml_dtypes

import concourse.bass as bass
import concourse.mybir as mybir
import concourse.tile as tile
from concourse import bacc
from concourse.bass_utils import run_bass_kernel_spmd

N_CORES = 8
B, N, D = 8, 4096, 512
C = 4096
TOK = B * N                  # 32768 tokens
TPC = TOK // N_CORES         # 4096 tokens per core (phase 1)
VPC = C // N_CORES           # 512 clusters per core (phase 2)
NTILES1 = TPC // 128         # 32 token tiles per core
CCH = C // 512               # 8 psum chunks of 512 clusters
KT = 12                      # 3 split terms x 4 k-tiles of 128
NT2 = 40                     # phase-2 token tiles per core (fixed, padded)
G2 = VPC // 128              # 4 cluster row groups
DECAY = 0.8
P = 128
F32 = mybir.dt.float32
BF16 = mybir.dt.bfloat16
I32 = mybir.dt.int32
U32 = mybir.dt.uint32

_cache = {}
LAST_RESULTS = {}


def _build_phase1():
    nc = bacc.Bacc("TRN2", target_bir_lowering=False, debug=False,
                   num_devices=N_CORES)
    xt_d = nc.dram_tensor("xt", [KT * P, TPC], BF16, kind="ExternalInput")
    et_d = nc.dram_tensor("et", [KT * P, C], BF16, kind="ExternalInput")
    emb_d = nc.dram_tensor("emb", [C, D], F32, kind="ExternalInput")
    dist_d = nc.dram_tensor("dist", [TPC, C], F32, kind="ExternalOutput")
    ind_d = nc.dram_tensor("ind", [TPC, 1], I32, kind="ExternalOutput")
    quant_d = nc.dram_tensor("quant", [TPC, D], F32, kind="ExternalOutput")

    xt_r = xt_d.ap().rearrange("(k p) t -> p k t", p=P)
    et_r = et_d.ap().rearrange("(k p) c -> p k c", p=P)

    with tile.TileContext(nc) as tc:
        with (
            tc.tile_pool(name="resident", bufs=1) as res_pool,
            tc.tile_pool(name="xt", bufs=3) as xt_pool,
            tc.tile_pool(name="dist", bufs=2) as dist_pool,
            tc.tile_pool(name="small", bufs=3) as small_pool,
            tc.tile_pool(name="psum", bufs=8, space="PSUM") as psum_pool,
        ):
            et_s = res_pool.tile([P, KT, C], BF16)
            nc.sync.dma_start(et_s[:], et_r)
            for t in range(NTILES1):
                xt_s = xt_pool.tile([P, KT, P], BF16, tag="xt")
                nc.sync.dma_start(xt_s[:], xt_r[:, :, t * P:(t + 1) * P])
                dist_s = dist_pool.tile([P, C], F32, tag="dist")
                for cc in range(CCH):
                    ps = psum_pool.tile([P, 512], F32, tag="ps")
                    for k in range(KT):
                        nc.tensor.matmul(
                            ps[:], xt_s[:, k],
                            et_s[:, k, cc * 512:(cc + 1) * 512],
                            start=(k == 0), stop=(k == KT - 1))
                    nc.scalar.copy(dist_s[:, cc * 512:(cc + 1) * 512], ps[:])
                nc.sync.dma_start(dist_d.ap()[t * P:(t + 1) * P], dist_s[:])
                mx = small_pool.tile([P, 8], F32, tag="mx")
                idx = small_pool.tile([P, 8], U32, tag="idx")
                nc.vector.max(out=mx[:], in_=dist_s[:])
                nc.vector.max_index(out=idx[:], in_max=mx[:], in_values=dist_s[:])
                idx32 = small_pool.tile([P, 1], I32, tag="idx32")
                nc.vector.tensor_copy(idx32[:], idx[:, 0:1])
                nc.sync.dma_start(ind_d.ap()[t * P:(t + 1) * P], idx32[:])
                q = small_pool.tile([P, D], F32, tag="q")
                nc.gpsimd.indirect_dma_start(
                    out=q[:], out_offset=None, in_=emb_d.ap(),
                    in_offset=bass.IndirectOffsetOnAxis(ap=idx32[:, :1], axis=0))
                nc.sync.dma_start(quant_d.ap()[t * P:(t + 1) * P], q[:])
    nc.compile()
    return nc


def _build_phase2():
    nc = bacc.Bacc("TRN2", target_bir_lowering=False, debug=False,
                   num_devices=N_CORES)
    x_d = nc.dram_tensor("x", [TOK, D], F32, kind="ExternalInput")
    # host layouts: gidxT [P, NT2] (token gather idx per tile column),
    # selT [P, NT2*P] (selection matrices, lhsT layout), goff [P, G2]
    # (stage row of cluster row g*128+p).
    gidx_d = nc.dram_tensor("gidx", [P, NT2], I32, kind="ExternalInput")
    sel_d = nc.dram_tensor("sel", [P, NT2 * P], F32, kind="ExternalInput")
    goff_d = nc.dram_tensor("goff", [P, G2], I32, kind="ExternalInput")
    cs_d = nc.dram_tensor("cs", [VPC], F32, kind="ExternalInput")
    ea_d = nc.dram_tensor("ea", [VPC, D], F32, kind="ExternalInput")
    stage_d = nc.dram_tensor("stage", [NT2 * P, D + 1], F32)
    ne_d = nc.dram_tensor("ne", [VPC, D], F32, kind="ExternalOutput")
    ncs_d = nc.dram_tensor("ncs", [VPC], F32, kind="ExternalOutput")

    with tile.TileContext(nc) as tc:
        with (
            tc.tile_pool(name="sbuf", bufs=3) as pool,
            tc.tile_pool(name="big", bufs=1) as big,
            tc.tile_pool(name="once", bufs=1) as once,
            tc.tile_pool(name="psum", bufs=3, space="PSUM") as psum,
        ):
            gidx_s = once.tile([P, NT2], I32)
            nc.sync.dma_start(gidx_s[:], gidx_d.ap())
            sel_s = once.tile([P, NT2, P], F32)
            nc.sync.dma_start(sel_s[:], sel_d.ap().rearrange(
                "p (t r) -> p t r", t=NT2))
            ones_s = once.tile([P, 1], F32)
            nc.vector.memset(ones_s[:], 1.0)
            for t in range(NT2):
                xg = pool.tile([P, D], F32, tag="xg")
                nc.gpsimd.indirect_dma_start(
                    out=xg[:], out_offset=None, in_=x_d.ap(),
                    in_offset=bass.IndirectOffsetOnAxis(
                        ap=gidx_s[:, t:t + 1], axis=0))
                comb_ps = psum.tile([P, D], F32, tag="comps")
                nc.tensor.matmul(comb_ps[:], sel_s[:, t], xg[:],
                                 start=True, stop=True)
                cnt_ps = psum.tile([P, 1], F32, tag="cntps")
                nc.tensor.matmul(cnt_ps[:], sel_s[:, t], ones_s[:],
                                 start=True, stop=True)
                comb = pool.tile([P, D + 1], F32, tag="comb")
                nc.scalar.copy(comb[:, :D], comb_ps[:])
                nc.scalar.copy(comb[:, D:D + 1], cnt_ps[:])
                nc.sync.dma_start(stage_d.ap()[t * P:(t + 1) * P], comb[:])
            goff_s = once.tile([P, G2], I32)
            nc.sync.dma_start(goff_s[:], goff_d.ap())
            tbl = big.tile([P, G2, D + 1], F32)
            for g in range(G2):
                nc.gpsimd.indirect_dma_start(
                    out=tbl[:, g], out_offset=None, in_=stage_d.ap(),
                    in_offset=bass.IndirectOffsetOnAxis(
                        ap=goff_s[:, g:g + 1], axis=0))
            # EMA + l2norm over the 512-cluster shard
            cs_s = pool.tile([P, G2], F32, tag="cs")
            nc.sync.dma_start(cs_s[:], cs_d.ap().rearrange("(g p) -> p g", p=P))
            ea_s = big.tile([P, G2, D], F32)
            nc.sync.dma_start(ea_s[:], ea_d.ap().rearrange("(g p) d -> p g d", p=P))
            ncs_s = pool.tile([P, G2], F32, tag="ncs")
            nc.vector.tensor_scalar_mul(ncs_s[:], cs_s[:], DECAY)
            cnt_sc = pool.tile([P, G2], F32, tag="cnts")
            nc.vector.tensor_scalar_mul(cnt_sc[:], tbl[:, :, D], 1.0 - DECAY)
            nc.vector.tensor_add(ncs_s[:], ncs_s[:], cnt_sc[:])
            nc.sync.dma_start(ncs_d.ap().rearrange("(g p) -> p g", p=P), ncs_s[:])
            nea = big.tile([P, G2, D], F32)
            nc.vector.tensor_scalar_mul(nea[:], ea_s[:], DECAY)
            tbl_sc = big.tile([P, G2, D], F32)
            nc.vector.tensor_scalar_mul(tbl_sc[:], tbl[:, :, :D], 1.0 - DECAY)
            nc.vector.tensor_add(nea[:], nea[:], tbl_sc[:])
            sq = big.tile([P, G2, D], F32)
            nc.vector.tensor_mul(sq[:], nea[:], nea[:])
            ss = pool.tile([P, G2], F32, tag="ss")
            nc.vector.reduce_sum(ss[:], sq[:], axis=mybir.AxisListType.X)
            nrm = pool.tile([P, G2], F32, tag="nrm")
            nc.scalar.sqrt(nrm[:], ss[:])
            nc.vector.tensor_scalar_max(nrm[:], nrm[:], 1e-12)
            inv = pool.tile([P, G2], F32, tag="inv")
            nc.vector.reciprocal(inv[:], nrm[:])
            ne_s = big.tile([P, G2, D], F32)
            nc.vector.tensor_tensor(out=ne_s[:], in0=nea[:],
                                    in1=inv[:, :, None].to_broadcast([P, G2, D]),
                                    op=mybir.AluOpType.mult)
            nc.sync.dma_start(ne_d.ap().rearrange("(g p) d -> p g d", p=P), ne_s[:])
    nc.compile()
    return nc


def _split3(a):
    """fp32 [K, M] -> bf16 hi/lo parts."""
    hi = a.astype(ml_dtypes.bfloat16)
    lo = (a - hi.astype(np.float32)).astype(ml_dtypes.bfloat16)
    return hi, lo


def _pack_shard(tok_sorted, clus_local):
    """Pack shard tokens (sorted by local cluster id 0..VPC) into NT2 tiles.

    Each tile owns a contiguous local-cluster window [base, base+R), R<=128,
    and holds all of those clusters' tokens (<=128 by construction).
    Returns gidxT [P, NT2] (token gather ids), selT [P, NT2*P] f32
    (selection matrices: sel[slot, r] = 1 if token slot belongs to cluster
    base+r), goff [P, G2] (stage row of each local cluster row).
    """
    n = len(tok_sorted)
    # token ranges per local cluster
    starts = np.searchsorted(clus_local, np.arange(VPC + 1))
    counts = np.diff(starts)
    assert counts.max(initial=0) <= P, "cluster too large for one tile"
    gidxT = np.zeros((P, NT2), np.int32)
    selT = np.zeros((P, NT2, P), np.float32)
    goff = np.zeros(VPC, np.int32)
    ti, base, pos = 0, 0, 0
    for c in range(VPC):
        cnt = int(counts[c])
        if pos + cnt > P or c - base >= P:
            ti += 1
            assert ti < NT2, "NT2 too small"
            base, pos = c, 0
        goff[c] = ti * P + (c - base)
        if cnt:
            gidxT[pos:pos + cnt, ti] = tok_sorted[starts[c]:starts[c + 1]]
            selT[pos:pos + cnt, ti, c - base] = 1.0
            pos += cnt
    return (gidxT, selT.reshape(P, NT2 * P),
            np.ascontiguousarray(goff.reshape(G2, P).T))


def kernel(x, embed, cluster_size, embed_avg):
    x = np.asarray(x, np.float32)
    embed = np.asarray(embed, np.float32)
    cluster_size = np.asarray(cluster_size, np.float32)
    embed_avg = np.asarray(embed_avg, np.float32)

    if "p1" not in _cache:
        _cache["p1"] = _build_phase1()
    if "p2" not in _cache:
        _cache["p2"] = _build_phase2()

    X = np.ascontiguousarray(x.reshape(TOK, D))
    E = np.ascontiguousarray(embed[0])

    xh, xl = _split3(np.ascontiguousarray(X.T))
    eh, el = _split3(np.ascontiguousarray(E.T))
    xt_full = np.concatenate([xh, xl, xh], 0)       # [1536, TOK]
    et_np = np.ascontiguousarray(np.concatenate([eh, eh, el], 0))  # [1536, C]

    in_maps1 = []
    for s in range(N_CORES):
        sl = slice(s * TPC, (s + 1) * TPC)
        in_maps1.append({
            "xt": np.ascontiguousarray(xt_full[:, sl]),
            "et": et_np,
            "emb": E,
        })
    r1 = run_bass_kernel_spmd(_cache["p1"], in_maps1, list(range(N_CORES)))
    LAST_RESULTS["p1"] = r1

    ind_full = np.concatenate(
        [r1.results[s]["ind"][:, 0] for s in range(N_CORES)])  # [TOK] int32

    order = np.argsort(ind_full, kind="stable").astype(np.int32)
    sorted_clus = ind_full[order]
    bounds = np.searchsorted(sorted_clus, np.arange(0, C + 1, VPC))
    in_maps2 = []
    for s in range(N_CORES):
        lo, hi = bounds[s], bounds[s + 1]
        gidxT, selT, goff = _pack_shard(order[lo:hi],
                                        sorted_clus[lo:hi] - s * VPC)
        in_maps2.append({
            "x": X,
            "gidx": gidxT,
            "sel": selT,
            "goff": goff,
            "cs": np.ascontiguousarray(cluster_size[0, s * VPC:(s + 1) * VPC]),
            "ea": np.ascontiguousarray(embed_avg[0, s * VPC:(s + 1) * VPC]),
        })
    r2 = run_bass_kernel_spmd(_cache["p2"], in_maps2, list(range(N_CORES)))
    LAST_RESULTS["p2"] = r2

    quantize = np.concatenate(
        [r1.results[s]["quant"] for s in range(N_CORES)]).reshape(B, N, D)
    embed_ind = ind_full.reshape(B, N).astype(np.int32)
    dist = np.concatenate(
        [r1.results[s]["dist"] for s in range(N_CORES)]).reshape(1, B, N, C)
    new_embed = np.concatenate(
        [r2.results[s]["ne"] for s in range(N_CORES)])[None]  # [1, C, D]
    new_cluster_size = np.concatenate(
        [r2.results[s]["ncs"] for s in range(N_CORES)])[None]  # [1, C]
    return quantize, embed_ind, dist, new_embed, new_cluster_size


# revision 6
# speedup vs baseline: 76229.1757x; 55592.3268x over previous
"""CosineSimCodebook (VQ) Trainium2 kernel, 8 NeuronCores.

Strategy:
  Phase 1 (token-data-parallel): each core takes 4096 tokens, computes
    dist = x @ embed^T via split-bf16 (hi*hi + lo*hi + hi*lo) matmuls on PE
    (near-fp32 accuracy at 3x bf16 cost), argmax via DVE Max8/MaxIndex,
    quantize via indirect-DMA gather of codebook rows.
  Host: sorts tokens by assigned cluster and builds routing metadata
    (gather lists + 0/1 selection matrices) — no arithmetic on host.
  Phase 2 (cluster-sharded): each core gathers its shard's token rows,
    sums same-cluster rows with a selection matmul into a dense per-tile
    window, stages windows to DRAM, gathers them back in cluster order,
    then applies the EMA update + l2norm.
    (cs_smoothed cancels inside l2norm, so laplace smoothing drops out of
    new_embed exactly; new_cluster_size does not depend on it.)
"""

import sys

sys.path.insert(0, "/opt/trn_rl_repo")

import numpy as np
import ml_dtypes

import concourse.bass as bass
import concourse.mybir as mybir
import concourse.tile as tile
from concourse import bacc
from concourse.bass_utils import run_bass_kernel_spmd

N_CORES = 8
B, N, D = 8, 4096, 512
C = 4096
TOK = B * N                  # 32768 tokens
TPC = TOK // N_CORES         # 4096 tokens per core (phase 1)
VPC = C // N_CORES           # 512 clusters per core (phase 2)
NTILES1 = TPC // 128         # 32 token tiles per core
CCH = C // 512               # 8 psum chunks of 512 clusters
KT = 12                      # 3 split terms x 4 k-tiles of 128
NT2 = 36                     # phase-2 token tiles per core (fixed, padded)
G2 = VPC // 128              # 4 cluster row groups
DECAY = 0.8
P = 128
F32 = mybir.dt.float32
BF16 = mybir.dt.bfloat16
I32 = mybir.dt.int32
U32 = mybir.dt.uint32

_cache = {}
LAST_RESULTS = {}


def _build_phase1():
    nc = bacc.Bacc("TRN2", target_bir_lowering=False, debug=False,
                   num_devices=N_CORES)
    xt_d = nc.dram_tensor("xt", [KT * P, TPC], BF16, kind="ExternalInput")
    et_d = nc.dram_tensor("et", [KT * P, C], BF16, kind="ExternalInput")
    emb_d = nc.dram_tensor("emb", [C, D], F32, kind="ExternalInput")
    dist_d = nc.dram_tensor("dist", [TPC, C], F32, kind="ExternalOutput")
    ind_d = nc.dram_tensor("ind", [TPC, 1], I32, kind="ExternalOutput")
    quant_d = nc.dram_tensor("quant", [TPC, D], F32, kind="ExternalOutput")

    xt_r = xt_d.ap().rearrange("(k p) t -> p k t", p=P)
    et_r = et_d.ap().rearrange("(k p) c -> p k c", p=P)

    with tile.TileContext(nc) as tc:
        with (
            tc.tile_pool(name="resident", bufs=1) as res_pool,
            tc.tile_pool(name="xt", bufs=3) as xt_pool,
            tc.tile_pool(name="dist", bufs=2) as dist_pool,
            tc.tile_pool(name="small", bufs=3) as small_pool,
            tc.tile_pool(name="psum", bufs=8, space="PSUM") as psum_pool,
        ):
            et_s = res_pool.tile([P, KT, C], BF16)
            # chunked load on the SWDGE queue so the first matmuls start as
            # soon as chunk 0 lands, while xt loads flow on the HWDGE queue
            for cc in range(CCH):
                nc.gpsimd.dma_start(et_s[:, :, cc * 512:(cc + 1) * 512],
                                    et_r[:, :, cc * 512:(cc + 1) * 512])
            for t in range(NTILES1):
                xt_s = xt_pool.tile([P, KT, P], BF16, tag="xt")
                nc.sync.dma_start(xt_s[:], xt_r[:, :, t * P:(t + 1) * P])
                dist_s = dist_pool.tile([P, C], F32, tag="dist")
                for cc in range(CCH):
                    ps = psum_pool.tile([P, 512], F32, tag="ps")
                    for k in range(KT):
                        nc.tensor.matmul(
                            ps[:], xt_s[:, k],
                            et_s[:, k, cc * 512:(cc + 1) * 512],
                            start=(k == 0), stop=(k == KT - 1))
                    nc.scalar.copy(dist_s[:, cc * 512:(cc + 1) * 512], ps[:])
                nc.sync.dma_start(dist_d.ap()[t * P:(t + 1) * P], dist_s[:])
                mx = small_pool.tile([P, 8], F32, tag="mx")
                idx = small_pool.tile([P, 8], U32, tag="idx")
                nc.vector.max(out=mx[:], in_=dist_s[:])
                nc.vector.max_index(out=idx[:], in_max=mx[:], in_values=dist_s[:])
                idx32 = small_pool.tile([P, 1], I32, tag="idx32")
                nc.vector.tensor_copy(idx32[:], idx[:, 0:1])
                nc.sync.dma_start(ind_d.ap()[t * P:(t + 1) * P], idx32[:])
                q = small_pool.tile([P, D], F32, tag="q")
                nc.gpsimd.indirect_dma_start(
                    out=q[:], out_offset=None, in_=emb_d.ap(),
                    in_offset=bass.IndirectOffsetOnAxis(ap=idx32[:, :1], axis=0))
                nc.sync.dma_start(quant_d.ap()[t * P:(t + 1) * P], q[:])
    nc.compile()
    return nc


def _build_phase2():
    nc = bacc.Bacc("TRN2", target_bir_lowering=False, debug=False,
                   num_devices=N_CORES)
    x_d = nc.dram_tensor("x", [TOK, D], F32, kind="ExternalInput")
    # host layouts: gidxT [P, NT2] (token gather idx per tile column),
    # selT [P, NT2*P] (selection matrices, lhsT layout), goff [P, G2]
    # (stage row of cluster row g*128+p).
    gidx_d = nc.dram_tensor("gidx", [P, NT2], I32, kind="ExternalInput")
    sel_d = nc.dram_tensor("sel", [P, NT2 * P], F32, kind="ExternalInput")
    goff_d = nc.dram_tensor("goff", [P, G2], I32, kind="ExternalInput")
    cs_d = nc.dram_tensor("cs", [VPC], F32, kind="ExternalInput")
    ea_d = nc.dram_tensor("ea", [VPC, D], F32, kind="ExternalInput")
    stage_d = nc.dram_tensor("stage", [NT2 * P, D + 1], F32)
    ne_d = nc.dram_tensor("ne", [VPC, D], F32, kind="ExternalOutput")
    ncs_d = nc.dram_tensor("ncs", [VPC], F32, kind="ExternalOutput")

    with tile.TileContext(nc) as tc:
        with (
            tc.tile_pool(name="sbuf", bufs=3) as pool,
            tc.tile_pool(name="big", bufs=1) as big,
            tc.tile_pool(name="once", bufs=1) as once,
            tc.tile_pool(name="psum", bufs=3, space="PSUM") as psum,
        ):
            gidx_s = once.tile([P, NT2], I32)
            nc.sync.dma_start(gidx_s[:], gidx_d.ap())
            sel_s = once.tile([P, NT2, P], F32)
            nc.sync.dma_start(sel_s[:], sel_d.ap().rearrange(
                "p (t r) -> p t r", t=NT2))
            ones_s = once.tile([P, 1], F32)
            nc.vector.memset(ones_s[:], 1.0)
            for t in range(NT2):
                xg = pool.tile([P, D], F32, tag="xg")
                nc.gpsimd.indirect_dma_start(
                    out=xg[:], out_offset=None, in_=x_d.ap(),
                    in_offset=bass.IndirectOffsetOnAxis(
                        ap=gidx_s[:, t:t + 1], axis=0))
                comb_ps = psum.tile([P, D], F32, tag="comps")
                nc.tensor.matmul(comb_ps[:], sel_s[:, t], xg[:],
                                 start=True, stop=True)
                cnt_ps = psum.tile([P, 1], F32, tag="cntps")
                nc.tensor.matmul(cnt_ps[:], sel_s[:, t], ones_s[:],
                                 start=True, stop=True)
                comb = pool.tile([P, D + 1], F32, tag="comb")
                nc.scalar.copy(comb[:, :D], comb_ps[:])
                nc.scalar.copy(comb[:, D:D + 1], cnt_ps[:])
                nc.sync.dma_start(stage_d.ap()[t * P:(t + 1) * P], comb[:])
            goff_s = once.tile([P, G2], I32)
            nc.sync.dma_start(goff_s[:], goff_d.ap())
            tbl = big.tile([P, G2, D + 1], F32)
            for g in range(G2):
                nc.gpsimd.indirect_dma_start(
                    out=tbl[:, g], out_offset=None, in_=stage_d.ap(),
                    in_offset=bass.IndirectOffsetOnAxis(
                        ap=goff_s[:, g:g + 1], axis=0))
            # EMA + l2norm over the 512-cluster shard
            cs_s = pool.tile([P, G2], F32, tag="cs")
            nc.sync.dma_start(cs_s[:], cs_d.ap().rearrange("(g p) -> p g", p=P))
            ea_s = big.tile([P, G2, D], F32)
            nc.sync.dma_start(ea_s[:], ea_d.ap().rearrange("(g p) d -> p g d", p=P))
            ncs_s = pool.tile([P, G2], F32, tag="ncs")
            nc.vector.tensor_scalar_mul(ncs_s[:], cs_s[:], DECAY)
            cnt_sc = pool.tile([P, G2], F32, tag="cnts")
            nc.vector.tensor_scalar_mul(cnt_sc[:], tbl[:, :, D], 1.0 - DECAY)
            nc.vector.tensor_add(ncs_s[:], ncs_s[:], cnt_sc[:])
            nc.sync.dma_start(ncs_d.ap().rearrange("(g p) -> p g", p=P), ncs_s[:])
            nea = big.tile([P, G2, D], F32)
            nc.vector.tensor_scalar_mul(nea[:], ea_s[:], DECAY)
            tbl_sc = big.tile([P, G2, D], F32)
            nc.vector.tensor_scalar_mul(tbl_sc[:], tbl[:, :, :D], 1.0 - DECAY)
            nc.vector.tensor_add(nea[:], nea[:], tbl_sc[:])
            sq = big.tile([P, G2, D], F32)
            nc.vector.tensor_mul(sq[:], nea[:], nea[:])
            ss = pool.tile([P, G2], F32, tag="ss")
            nc.vector.reduce_sum(ss[:], sq[:], axis=mybir.AxisListType.X)
            nrm = pool.tile([P, G2], F32, tag="nrm")
            nc.scalar.sqrt(nrm[:], ss[:])
            nc.vector.tensor_scalar_max(nrm[:], nrm[:], 1e-12)
            inv = pool.tile([P, G2], F32, tag="inv")
            nc.vector.reciprocal(inv[:], nrm[:])
            ne_s = big.tile([P, G2, D], F32)
            nc.vector.tensor_tensor(out=ne_s[:], in0=nea[:],
                                    in1=inv[:, :, None].to_broadcast([P, G2, D]),
                                    op=mybir.AluOpType.mult)
            nc.sync.dma_start(ne_d.ap().rearrange("(g p) d -> p g d", p=P), ne_s[:])
    nc.compile()
    return nc


def _split3(a):
    """fp32 [K, M] -> bf16 hi/lo parts."""
    hi = a.astype(ml_dtypes.bfloat16)
    lo = (a - hi.astype(np.float32)).astype(ml_dtypes.bfloat16)
    return hi, lo


def _pack_shard(tok_sorted, clus_local):
    """Pack shard tokens (sorted by local cluster id 0..VPC) into NT2 tiles.

    Each tile owns a contiguous local-cluster window [base, base+R), R<=128,
    and holds all of those clusters' tokens (<=128 by construction).
    Returns gidxT [P, NT2] (token gather ids), selT [P, NT2*P] f32
    (selection matrices: sel[slot, r] = 1 if token slot belongs to cluster
    base+r), goff [P, G2] (stage row of each local cluster row).
    """
    n = len(tok_sorted)
    # token ranges per local cluster
    starts = np.searchsorted(clus_local, np.arange(VPC + 1))
    counts = np.diff(starts)
    assert counts.max(initial=0) <= P, "cluster too large for one tile"
    gidxT = np.zeros((P, NT2), np.int32)
    selT = np.zeros((P, NT2, P), np.float32)
    goff = np.zeros(VPC, np.int32)
    ti, base, pos = 0, 0, 0
    for c in range(VPC):
        cnt = int(counts[c])
        if pos + cnt > P or c - base >= P:
            ti += 1
            assert ti < NT2, "NT2 too small"
            base, pos = c, 0
        goff[c] = ti * P + (c - base)
        if cnt:
            gidxT[pos:pos + cnt, ti] = tok_sorted[starts[c]:starts[c + 1]]
            selT[pos:pos + cnt, ti, c - base] = 1.0
            pos += cnt
    return (gidxT, selT.reshape(P, NT2 * P),
            np.ascontiguousarray(goff.reshape(G2, P).T))


def kernel(x, embed, cluster_size, embed_avg):
    x = np.asarray(x, np.float32)
    embed = np.asarray(embed, np.float32)
    cluster_size = np.asarray(cluster_size, np.float32)
    embed_avg = np.asarray(embed_avg, np.float32)

    if "p1" not in _cache:
        _cache["p1"] = _build_phase1()
    if "p2" not in _cache:
        _cache["p2"] = _build_phase2()

    X = np.ascontiguousarray(x.reshape(TOK, D))
    E = np.ascontiguousarray(embed[0])

    xh, xl = _split3(np.ascontiguousarray(X.T))
    eh, el = _split3(np.ascontiguousarray(E.T))
    xt_full = np.concatenate([xh, xl, xh], 0)       # [1536, TOK]
    et_np = np.ascontiguousarray(np.concatenate([eh, eh, el], 0))  # [1536, C]

    in_maps1 = []
    for s in range(N_CORES):
        sl = slice(s * TPC, (s + 1) * TPC)
        in_maps1.append({
            "xt": np.ascontiguousarray(xt_full[:, sl]),
            "et": et_np,
            "emb": E,
        })
    r1 = run_bass_kernel_spmd(_cache["p1"], in_maps1, list(range(N_CORES)))
    LAST_RESULTS["p1"] = r1

    ind_full = np.concatenate(
        [r1.results[s]["ind"][:, 0] for s in range(N_CORES)])  # [TOK] int32

    order = np.argsort(ind_full, kind="stable").astype(np.int32)
    sorted_clus = ind_full[order]
    bounds = np.searchsorted(sorted_clus, np.arange(0, C + 1, VPC))
    in_maps2 = []
    for s in range(N_CORES):
        lo, hi = bounds[s], bounds[s + 1]
        gidxT, selT, goff = _pack_shard(order[lo:hi],
                                        sorted_clus[lo:hi] - s * VPC)
        in_maps2.append({
            "x": X,
            "gidx": gidxT,
            "sel": selT,
            "goff": goff,
            "cs": np.ascontiguousarray(cluster_size[0, s * VPC:(s + 1) * VPC]),
            "ea": np.ascontiguousarray(embed_avg[0, s * VPC:(s + 1) * VPC]),
        })
    r2 = run_bass_kernel_spmd(_cache["p2"], in_maps2, list(range(N_CORES)))
    LAST_RESULTS["p2"] = r2

    quantize = np.concatenate(
        [r1.results[s]["quant"] for s in range(N_CORES)]).reshape(B, N, D)
    embed_ind = ind_full.reshape(B, N).astype(np.int32)
    dist = np.concatenate(
        [r1.results[s]["dist"] for s in range(N_CORES)]).reshape(1, B, N, C)
    new_embed = np.concatenate(
        [r2.results[s]["ne"] for s in range(N_CORES)])[None]  # [1, C, D]
    new_cluster_size = np.concatenate(
        [r2.results[s]["ncs"] for s in range(N_CORES)])[None]  # [1, C]
    return quantize, embed_ind, dist, new_embed, new_cluster_size
